# revision 2
# baseline (speedup 1.0000x reference)
# Trainium2 Bass kernel for the Tacotron-style decoder (2-layer LSTM, B=32,
# T=1000). Strategy: data-parallel over batch, 4 sequences per NeuronCore.
# All compute is local per core (no collectives):
#   Ph1  transpose memory + shifted mels to channel-major
#   Ph2  prenet (2x matmul+relu), channel-major
#   Ph3  xg0 = w_ih0 @ x + b   (batched over all timesteps)
#   Ph4  layer-0 LSTM recurrence
#   Ph5  xg1 = w_ih1 @ h0 + b  (batched)
#   Ph6  layer-1 LSTM recurrence
#   Ph7  projection out = W_proj @ [h1; mem] + b
#
# Recurrence schedule (the hot 92%): per step, 256 LDWEIGHTS+MATMUL pairs
# (32 gate m-tiles x 8 k-tiles, N=4) accumulate into two psum tiles (halves
# A = gate blocks 0-3, B = blocks 4-7).  The matmuls are ordered
#   P1: all mm x k0-3   P2: mmA x k4-7   [cellA]   P3: mmB x k4-7   [cellB]
# so that each step's P1 only needs cellA of the previous step (ready) and
# P2 only needs cellB (computed during the previous P1).  The LSTM cell is
# done with wide strided ops (5 DVE + 3 ACT per half-step instead of 16
# narrow ops per block), writing h directly into the bf16 tile the next
# step's matmuls read, so the PE stream never stalls on the cell chain.
import functools
import numpy as np
import ml_dtypes

B, T, A, M = 32, 1000, 512, 80
P, H = 256, 1024
NCORES = 8
BC = B // NCORES            # 4 sequences per core
F = BC * T                  # 4000 frames per core, frame f = t*BC + b
G4 = 4 * H                  # 4096 gate rows
NBLK = H // 128             # 8 channel blocks
SBLK = 20                   # recurrence steps per hardware-loop iteration
# gate order used on-chip: i, f, o, g  (PyTorch order is i, f, g, o)
GORDER = (0, 1, 3, 2)
NCHUNK = 8                  # frame chunks for batched GEMMs
FCH = F // NCHUNK           # 500 frames per chunk
WHH_FP8 = False             # recurrent weights in fp8e4 (FWL loads 4/cycle)


def _arrange_cols(wt):
    """wt [K, 4096] (= w.T, PyTorch gate order i,f,g,o on columns) ->
    columns reordered to m-index = blk*4 + gi with gi over (i,f,o,g)."""
    cols = []
    for blk in range(NBLK):
        for go in GORDER:
            cols.append(wt[:, go * H + blk * 128: go * H + (blk + 1) * 128])
    return np.ascontiguousarray(np.concatenate(cols, axis=1))


def _arrange_vec(b):
    return _arrange_cols(b.reshape(1, G4))[0]


@functools.lru_cache(maxsize=1)
def _build():
    import concourse.bacc as bacc
    import concourse.mybir as mybir
    from concourse import tile
    import concourse.bass as bass

    dt = mybir.dt
    whh_dt = dt.float8e4 if WHH_FP8 else dt.bfloat16
    nc = bacc.Bacc(None)

    # ---------------- I/O ----------------
    mem_f = nc.declare_dram_parameter("mem_f", [F, A], dt.float32, isOutput=False)
    y_f = nc.declare_dram_parameter("y_f", [F, M], dt.float32, isOutput=False)
    ident = nc.declare_dram_parameter("ident", [128, 128], dt.float32, isOutput=False)
    w1t = nc.declare_dram_parameter("w1t", [M, P], dt.float32, isOutput=False)
    w2t = nc.declare_dram_parameter("w2t", [P, P], dt.float32, isOutput=False)
    wih0t = nc.declare_dram_parameter("wih0t", [P + A, G4], dt.float32, isOutput=False)
    whh0t = nc.declare_dram_parameter("whh0t", [H, G4], whh_dt, isOutput=False)
    wih1t = nc.declare_dram_parameter("wih1t", [H, G4], dt.bfloat16, isOutput=False)
    whh1t = nc.declare_dram_parameter("whh1t", [H, G4], whh_dt, isOutput=False)
    b0in = nc.declare_dram_parameter("b0in", [1, G4], dt.float32, isOutput=False)
    b1in = nc.declare_dram_parameter("b1in", [1, G4], dt.float32, isOutput=False)
    wpt_h = nc.declare_dram_parameter("wpt_h", [H, M], dt.bfloat16, isOutput=False)
    wpt_m = nc.declare_dram_parameter("wpt_m", [A, M], dt.float32, isOutput=False)
    bpin = nc.declare_dram_parameter("bpin", [1, M], dt.float32, isOutput=False)
    outT = nc.declare_dram_parameter("outT", [M, F], dt.float32, isOutput=True)

    # ---------------- internal DRAM ----------------
    memT_d = nc.dram_tensor("memT_d", [A, F], dt.float32)
    xg0T = nc.dram_tensor("xg0T", [G4, F], dt.float32)
    h0T = nc.dram_tensor("h0T", [H, F], dt.bfloat16)
    xg1T = nc.dram_tensor("xg1T", [G4, F], dt.float32)
    h1T = nc.dram_tensor("h1T", [H, F], dt.bfloat16)

    FT = (F + 127) // 128  # 32 frame tiles (31 full + 1 of 32 rows)

    def ftrows(ft):
        return min(128, F - ft * 128)

    ACT = mybir.ActivationFunctionType

    with tile.TileContext(nc) as tc:
        with tc.tile_pool(name="const", bufs=1) as cpool:
            idsb = cpool.tile([128, 128], dt.float32, name="idsb")
            nc.sync.dma_start(idsb[:], ident[:])
            b0sb = cpool.tile([128, 32], dt.float32, name="b0sb")
            b1sb = cpool.tile([128, 32], dt.float32, name="b1sb")
            bpsb = cpool.tile([M, 1], dt.float32, name="bpsb")
            # bias column m at b*sb[:, m]
            nc.sync.dma_start(b0sb[:], b0in[:].rearrange("o (m p) -> (o p) m", p=128))
            nc.sync.dma_start(b1sb[:], b1in[:].rearrange("o (m p) -> (o p) m", p=128))
            nc.sync.dma_start(bpsb[:], bpin[:].rearrange("o (m u) -> (o m) u", u=1))

            # persistent channel-major activations
            with tc.tile_pool(name="actsb", bufs=1) as apool:
                prevT = apool.tile([M, F], dt.float32, name="prevT")
                p2T = apool.tile([128, 2 * F], dt.float32, name="p2T")

                # ---------- Ph1: transposes ----------
                with tc.tile_pool(name="tr", bufs=3) as trp, \
                     tc.tile_pool(name="trps", bufs=2, space="PSUM") as trps:
                    for ft in range(FT):
                        r = ftrows(ft)
                        # shifted mels -> prevT
                        yin = trp.tile([128, M], dt.float32, name="yin", tag="yin")
                        if ft == 0:
                            nc.gpsimd.memset(yin[:, :], 0.0)
                            nc.sync.dma_start(yin[BC:r, :], y_f[0:r - BC, :])
                        else:
                            nc.sync.dma_start(yin[0:r, :], y_f[ft * 128 - BC: ft * 128 - BC + r, :])
                        yps = trps.tile([M, 128], dt.float32, name="yps", tag="yps")
                        nc.tensor.transpose(yps[:, 0:r], yin[0:r, :], idsb[0:r, 0:r])
                        nc.scalar.copy(prevT[:, ft * 128: ft * 128 + r], yps[:, 0:r])
                        # memory -> memT (4 column blocks)
                        for cb in range(A // 128):
                            min_ = trp.tile([128, 128], dt.float32, name="min_", tag="min")
                            nc.sync.dma_start(min_[0:r, :], mem_f[ft * 128: ft * 128 + r, cb * 128:(cb + 1) * 128])
                            mps = trps.tile([128, 128], dt.float32, name="mps", tag="mps")
                            nc.tensor.transpose(mps[:, 0:r], min_[0:r, :], idsb[0:r, 0:r])
                            mrow = trp.tile([128, 128], dt.float32, name="mrow", tag="mrow")
                            nc.scalar.copy(mrow[:, 0:r], mps[:, 0:r])
                            nc.sync.dma_start(memT_d[cb * 128:(cb + 1) * 128, ft * 128: ft * 128 + r], mrow[:, 0:r])

                # ---------- Ph2: prenet ----------
                with tc.tile_pool(name="pn", bufs=2) as pnp, \
                     tc.tile_pool(name="pnps", bufs=2, space="PSUM") as pnps:
                    w1sb = pnp.tile([M, P], dt.float32, name="w1sb")
                    nc.sync.dma_start(w1sb[:], w1t[:])
                    p1T = pnp.tile([128, 2 * F], dt.float32, name="p1T")
                    for m in range(P // 128):
                        for n in range(NCHUNK):
                            ps = pnps.tile([128, FCH], dt.float32, name="pnps1", tag=f"pn{n % 4}")
                            nc.tensor.matmul(ps[:], w1sb[:, m * 128:(m + 1) * 128],
                                             prevT[:, n * FCH:(n + 1) * FCH], start=True, stop=True)
                            nc.scalar.activation(p1T[:, m * F + n * FCH: m * F + (n + 1) * FCH], ps[:], ACT.Relu)
                    w2sb = pnp.tile([128, 2 * P], dt.float32, name="w2sb")
                    for k in range(P // 128):
                        nc.sync.dma_start(w2sb[:, k * P:(k + 1) * P], w2t[k * 128:(k + 1) * 128, :])
                    for m in range(P // 128):
                        for n in range(NCHUNK):
                            ps = pnps.tile([128, FCH], dt.float32, name="pnps2", tag=f"pn{n % 4}")
                            for k in range(P // 128):
                                nc.tensor.matmul(ps[:], w2sb[:, k * P + m * 128: k * P + (m + 1) * 128],
                                                 p1T[:, k * F + n * FCH: k * F + (n + 1) * FCH],
                                                 start=(k == 0), stop=(k == 1))
                            nc.scalar.activation(p2T[:, m * F + n * FCH: m * F + (n + 1) * FCH], ps[:], ACT.Relu)

                # ---------- Ph3: xg0 ----------
                # rhs K-tiles: 2 from p2T, 4 from memT (SBUF-resident copy)
                KX = 6
                with tc.tile_pool(name="x0", bufs=2) as x0p, \
                     tc.tile_pool(name="x0ps", bufs=1, space="PSUM") as x0ps:
                    memTsb = x0p.tile([128, 4 * F], dt.float32, name="memTsb")
                    for cb in range(4):
                        nc.sync.dma_start(memTsb[:, cb * F:(cb + 1) * F], memT_d[cb * 128:(cb + 1) * 128, :])

                    def x_rhs(k, n):
                        if k < 2:
                            return p2T[:, k * F + n * FCH: k * F + (n + 1) * FCH]
                        cb = k - 2
                        return memTsb[:, cb * F + n * FCH: cb * F + n * FCH + FCH]

                    for m in range(32):
                        wtile = x0p.tile([128, 6 * 128], dt.float32, name="wtile", tag="w0t")
                        for k in range(KX):
                            nc.sync.dma_start(
                                wtile[:, k * 128:(k + 1) * 128],
                                wih0t[k * 128:(k + 1) * 128, m * 128:(m + 1) * 128])
                        pss = []
                        for n in range(NCHUNK):
                            ps = x0ps.tile([128, FCH], dt.float32, name="x0psn", tag=f"x0{n}")
                            pss.append(ps)
                        for k in range(KX):
                            for n in range(NCHUNK):
                                nc.tensor.matmul(pss[n][:], wtile[:, k * 128:(k + 1) * 128], x_rhs(k, n),
                                                 start=(k == 0), stop=(k == KX - 1))
                        for n in range(NCHUNK):
                            otile = x0p.tile([128, FCH], dt.float32, name="otile", tag="x0o")
                            nc.vector.tensor_scalar_add(otile[:], pss[n][:], b0sb[:, m:m + 1])
                            nc.sync.dma_start(xg0T[m * 128:(m + 1) * 128, n * FCH:(n + 1) * FCH], otile[:])

            # ---------- recurrence helper ----------
            def recurrence(whhT_in, xgT_d, hT_out):
                NB = T // SBLK
                with tc.tile_pool(name="rc", bufs=1) as rp, \
                     tc.tile_pool(name="rcx", bufs=2) as rxp, \
                     tc.tile_pool(name="rcps", bufs=1, space="PSUM") as rps, \
                     tc.tile_pool(name="rct", bufs=2) as rtp:
                    whsb = rp.tile([128, 8 * G4], whh_dt, name="whsb")
                    for k in range(8):
                        nc.sync.dma_start(whsb[:, k * G4:(k + 1) * G4], whhT_in[k * 128:(k + 1) * 128, :])

                    def wh(k, mm):
                        return whsb[:, k * G4 + mm * 128: k * G4 + (mm + 1) * 128]

                    # h for all SBLK steps, split by half (gate blocks 0-3 / 4-7)
                    # col s*16 + blk*4 + b;   step s writes slot s, reads slot s-1
                    hA = rp.tile([128, SBLK * 16], dt.bfloat16, name="hA")
                    hB = rp.tile([128, SBLK * 16], dt.bfloat16, name="hB")
                    cA = [rp.tile([128, 16], dt.float32, name=f"cA{i}") for i in range(2)]
                    cB = [rp.tile([128, 16], dt.float32, name=f"cB{i}") for i in range(2)]
                    nc.gpsimd.memset(hA[:], 0.0)
                    nc.gpsimd.memset(hB[:], 0.0)
                    for i in range(2):
                        nc.gpsimd.memset(cA[i][:], 0.0)
                        nc.gpsimd.memset(cB[i][:], 0.0)
                    # full-bank psum tiles: [0:64] used (16 mm-tiles x 4 batch)
                    psA = [rps.tile([128, 512], dt.float32, name=f"psA{i}") for i in range(2)]
                    psB = [rps.tile([128, 512], dt.float32, name=f"psB{i}") for i in range(2)]

                    def cell(ps, xg3, c_in, c_out, h_t, s):
                        # ps: [128, 64] psum (16 mm x 4b); xg3 view [128,16,4]
                        # layout per gate-block blk(4): gi in (i,f,o,g) x b(4)
                        zt = rtp.tile([128, 64], dt.float32, name="zt", tag=f"zt{s % 2}")
                        z3 = zt[:].rearrange("p (m b) -> p m b", m=16)
                        nc.vector.tensor_add(z3, ps.rearrange("p (m b) -> p m b", m=16), xg3)
                        z4 = zt[:].rearrange("p (blk gi b) -> p blk gi b", blk=4, gi=4)
                        st = rtp.tile([128, 48], dt.float32, name="st", tag=f"st{s % 2}")
                        s4 = st[:].rearrange("p (blk gi b) -> p blk gi b", blk=4, gi=3)
                        nc.scalar.activation(s4, z4[:, :, 0:3, :], ACT.Sigmoid)
                        gt = rtp.tile([128, 16], dt.float32, name="gt", tag=f"gt{s % 2}")
                        g3 = gt[:].rearrange("p (blk b) -> p blk b", blk=4)
                        nc.scalar.activation(g3, z4[:, :, 3, :], ACT.Tanh)
                        aa = rtp.tile([128, 16], dt.float32, name="aa", tag=f"aa{s % 2}")
                        a3 = aa[:].rearrange("p (blk b) -> p blk b", blk=4)
                        c3i = c_in[:].rearrange("p (blk b) -> p blk b", blk=4)
                        nc.vector.tensor_mul(a3, s4[:, :, 1, :], c3i)
                        bb = rtp.tile([128, 16], dt.float32, name="bb", tag=f"bb{s % 2}")
                        b3 = bb[:].rearrange("p (blk b) -> p blk b", blk=4)
                        nc.vector.tensor_mul(b3, s4[:, :, 0, :], g3)
                        nc.vector.tensor_add(c_out[:], aa[:], bb[:])
                        tcx = rtp.tile([128, 16], dt.float32, name="tcx", tag=f"tc{s % 2}")
                        t3 = tcx[:].rearrange("p (blk b) -> p blk b", blk=4)
                        nc.scalar.activation(t3, c_out[:].rearrange("p (blk b) -> p blk b", blk=4), ACT.Tanh)
                        h3 = h_t.rearrange("p (blk b) -> p blk b", blk=4)
                        nc.vector.tensor_mul(h3, s4[:, :, 2, :], t3)

                    with tc.For_i(0, NB, 1, hint_engines=(mybir.EngineType.PE,
                                                          mybir.EngineType.DVE,
                                                          mybir.EngineType.Activation)) as bi:
                        # xg for SBLK steps: col rr*(4*SBLK) + s*4 + b
                        xgsb = rxp.tile([128, 32 * 4 * SBLK], dt.float32, name="xgsb", tag="xgsb")
                        for rr in range(32):
                            nc.sync.dma_start(
                                xgsb[:, rr * 4 * SBLK:(rr + 1) * 4 * SBLK],
                                xgT_d[rr * 128:(rr + 1) * 128, bass.ts(bi, 4 * SBLK)])
                        xgv = xgsb[:].rearrange("p (rr sb) -> p rr sb", rr=32)

                        for s in range(SBLK):
                            q = s % 2
                            pv = (s - 1) % SBLK
                            pA, pB = psA[q], psB[q]
                            # P1: all mm, k 0-3 (rhs = hA slot s-1)
                            for mm in range(32):
                                dst = (pA if mm < 16 else pB)[:, (mm % 16) * 4:(mm % 16) * 4 + 4]
                                for k in range(4):
                                    nc.tensor.matmul(dst, wh(k, mm),
                                                     hA[:, pv * 16 + k * 4: pv * 16 + k * 4 + 4],
                                                     start=(k == 0), stop=False)
                            # P2: mm 0-15, k 4-7 (rhs = hB slot s-1)
                            for mm in range(16):
                                dst = pA[:, mm * 4: mm * 4 + 4]
                                for k in range(4, 8):
                                    nc.tensor.matmul(dst, wh(k, mm),
                                                     hB[:, pv * 16 + (k - 4) * 4: pv * 16 + (k - 4) * 4 + 4],
                                                     start=False, stop=(k == 7))
                            # cell A (gate blocks 0-3)
                            cell(pA[:, 0:64], xgv[:, 0:16, s * 4:(s + 1) * 4],
                                 cA[1 - q], cA[q], hA[:, s * 16:(s + 1) * 16], s)
                            # P3: mm 16-31, k 4-7
                            for mm in range(16, 32):
                                dst = pB[:, (mm - 16) * 4:(mm - 16) * 4 + 4]
                                for k in range(4, 8):
                                    nc.tensor.matmul(dst, wh(k, mm),
                                                     hB[:, pv * 16 + (k - 4) * 4: pv * 16 + (k - 4) * 4 + 4],
                                                     start=False, stop=(k == 7))
                            # cell B (gate blocks 4-7)
                            cell(pB[:, 0:64], xgv[:, 16:32, s * 4:(s + 1) * 4],
                                 cB[1 - q], cB[q], hB[:, s * 16:(s + 1) * 16], s)

                        # write out h for all SBLK steps (8 DMAs, one per block)
                        for half, ht in ((0, hA), (1, hB)):
                            hv = ht[:].rearrange("p (s bb) -> p s bb", s=SBLK)
                            for blk in range(4):
                                gb = half * 4 + blk
                                nc.sync.dma_start(
                                    hT_out[gb * 128:(gb + 1) * 128, bass.ts(bi, 4 * SBLK)],
                                    hv[:, :, blk * 4:(blk + 1) * 4])

            # ---------- Ph4: layer-0 recurrence ----------
            recurrence(whh0t, xg0T, h0T)

            # ---------- Ph5: xg1 ----------
            with tc.tile_pool(name="x1", bufs=1) as x1p, \
                 tc.tile_pool(name="x1w", bufs=2) as x1wp, \
                 tc.tile_pool(name="x1ps", bufs=1, space="PSUM") as x1ps:
                h0sb = x1p.tile([128, 8 * F], dt.bfloat16, name="h0sb")
                for k in range(8):
                    nc.sync.dma_start(h0sb[:, k * F:(k + 1) * F], h0T[k * 128:(k + 1) * 128, :])
                for m in range(32):
                    wtile = x1wp.tile([128, 8 * 128], dt.bfloat16, name="w1tile", tag="w1t")
                    for k in range(8):
                        nc.sync.dma_start(
                            wtile[:, k * 128:(k + 1) * 128],
                            wih1t[k * 128:(k + 1) * 128, m * 128:(m + 1) * 128])
                    pss = []
                    for n in range(NCHUNK):
                        ps = x1ps.tile([128, FCH], dt.float32, name="x1psn", tag=f"x1{n}")
                        pss.append(ps)
                    for k in range(8):
                        for n in range(NCHUNK):
                            nc.tensor.matmul(pss[n][:], wtile[:, k * 128:(k + 1) * 128],
                                             h0sb[:, k * F + n * FCH: k * F + n * FCH + FCH],
                                             start=(k == 0), stop=(k == 7))
                    for n in range(NCHUNK):
                        otile = x1wp.tile([128, FCH], dt.float32, name="o1tile", tag="x1o")
                        nc.vector.tensor_scalar_add(otile[:], pss[n][:], b1sb[:, m:m + 1])
                        nc.sync.dma_start(xg1T[m * 128:(m + 1) * 128, n * FCH:(n + 1) * FCH], otile[:])

            # ---------- Ph6: layer-1 recurrence ----------
            recurrence(whh1t, xg1T, h1T)

            # ---------- Ph7: projection ----------
            with tc.tile_pool(name="pj", bufs=1) as pjp, \
                 tc.tile_pool(name="pjw", bufs=2) as pjwp, \
                 tc.tile_pool(name="pjps", bufs=2, space="PSUM") as pjps:
                h1sb = pjp.tile([128, 8 * F], dt.bfloat16, name="h1sb")
                for k in range(8):
                    nc.sync.dma_start(h1sb[:, k * F:(k + 1) * F], h1T[k * 128:(k + 1) * 128, :])
                memTsb2 = pjp.tile([128, 4 * F], dt.float32, name="memTsb2")
                for cb in range(4):
                    nc.sync.dma_start(memTsb2[:, cb * F:(cb + 1) * F], memT_d[cb * 128:(cb + 1) * 128, :])
                wphsb = pjp.tile([128, 8 * M], dt.bfloat16, name="wphsb")
                for k in range(8):
                    nc.sync.dma_start(wphsb[:, k * M:(k + 1) * M], wpt_h[k * 128:(k + 1) * 128, :])
                wpmsb = pjp.tile([128, 4 * M], dt.float32, name="wpmsb")
                for k in range(4):
                    nc.sync.dma_start(wpmsb[:, k * M:(k + 1) * M], wpt_m[k * 128:(k + 1) * 128, :])
                for n in range(NCHUNK):
                    ps = pjps.tile([M, FCH], dt.float32, name="pjpsn", tag=f"pj{n % 4}")
                    for k in range(8):
                        nc.tensor.matmul(ps[:], wphsb[:, k * M:(k + 1) * M],
                                         h1sb[:, k * F + n * FCH: k * F + n * FCH + FCH],
                                         start=(k == 0), stop=False)
                    for cb in range(4):
                        nc.tensor.matmul(ps[:], wpmsb[:, cb * M:(cb + 1) * M],
                                         memTsb2[:, cb * F + n * FCH: cb * F + n * FCH + FCH],
                                         start=False, stop=(cb == 3))
                    otile = pjwp.tile([M, FCH], dt.float32, name="pjo", tag="pjo")
                    nc.vector.tensor_scalar_add(otile[:], ps[:], bpsb[:, 0:1])
                    nc.sync.dma_start(outT[:, n * FCH:(n + 1) * FCH], otile[:])

    nc.finalize()
    return nc


def kernel(memory, y_mels, W1, W2, w_ih0, w_hh0, b_ih0, b_hh0,
           w_ih1, w_hh1, b_ih1, b_hh1, W_proj, b_proj, _trace=False):
    from concourse.bass_utils import run_bass_kernel_spmd

    nc = _build()
    bf16 = ml_dtypes.bfloat16
    whh_np = ml_dtypes.float8_e4m3 if WHH_FP8 else bf16
    ident = np.eye(128, dtype=np.float32)
    w1t = np.ascontiguousarray(W1.T.astype(np.float32))
    w2t = np.ascontiguousarray(W2.T.astype(np.float32))
    wih0t = _arrange_cols(w_ih0.T.astype(np.float32))
    whh0t = _arrange_cols(w_hh0.T.astype(np.float32)).astype(whh_np)
    wih1t = _arrange_cols(w_ih1.T.astype(np.float32)).astype(bf16)
    whh1t = _arrange_cols(w_hh1.T.astype(np.float32)).astype(whh_np)
    b0 = _arrange_vec((b_ih0 + b_hh0).astype(np.float32)).reshape(1, G4)
    b1 = _arrange_vec((b_ih1 + b_hh1).astype(np.float32)).reshape(1, G4)
    wpt = W_proj.T.astype(np.float32)
    wpt_h = np.ascontiguousarray(wpt[:H]).astype(bf16)
    wpt_m = np.ascontiguousarray(wpt[H:])
    bp = b_proj.astype(np.float32).reshape(1, M)

    in_maps = []
    for c in range(NCORES):
        mem_c = memory[c * BC:(c + 1) * BC]          # [4, 1000, 512]
        y_c = y_mels[c * BC:(c + 1) * BC]            # [4, 1000, 80]
        mem_fc = np.ascontiguousarray(mem_c.transpose(1, 0, 2).reshape(F, A)).astype(np.float32)
        y_fc = np.ascontiguousarray(y_c.transpose(1, 0, 2).reshape(F, M)).astype(np.float32)
        in_maps.append(dict(
            mem_f=mem_fc, y_f=y_fc, ident=ident, w1t=w1t, w2t=w2t,
            wih0t=wih0t, whh0t=whh0t, wih1t=wih1t, whh1t=whh1t,
            b0in=b0, b1in=b1, wpt_h=wpt_h, wpt_m=wpt_m, bpin=bp))

    res = run_bass_kernel_spmd(nc, in_maps, core_ids=list(range(NCORES)), trace=_trace)
    outs = []
    for c in range(NCORES):
        oT = res.results[c]["outT"]                  # [80, 4000]
        outs.append(oT.reshape(M, T, BC).transpose(2, 1, 0))  # [4, 1000, 80]
    full = np.concatenate(outs, axis=0).astype(np.float32)
    if _trace:
        kernel.last_exec_time_ns = res.exec_time_ns
    return full


# revision 3
# speedup vs baseline: 1.5299x; 1.5299x over previous
# Trainium2 Bass kernel for the Tacotron-style decoder (2-layer LSTM, B=32,
# T=1000). Strategy: data-parallel over batch, 4 sequences per NeuronCore.
# All compute is local per core (no collectives).  Host pre-transposes the
# inputs to channel-major, so the device phases are:
#   Ph2  prenet (2x matmul+relu), channel-major
#   Ph3  xg0 = w_ih0 @ x + b   (batched over all timesteps)
#   Ph4  layer-0 LSTM recurrence
#   Ph5  xg1 = w_ih1 @ h0 + b  (batched)
#   Ph6  layer-1 LSTM recurrence
#   Ph7  projection out = W_proj @ [h1; mem] + b
#
# Recurrence (the hot 92%): per step, 256 LDWEIGHTS+MATMUL pairs (32 gate
# m-tiles x 8 k-tiles, N=4 batch).  Gate m-tiles are ordered gi-major per
# half (half A = channel blocks 0-3, B = 4-7; within a half: i,f,o,g x blk)
# so every LSTM-cell op is a contiguous 2D tile op.  Accumulation is split
# into psLo (k0-3) / psHi (k4-7) per-mm contiguous groups; schedule
#   P1: all mm kLo | P2: mmA kHi | cellA | P3: mmB kHi | cellB
# lets each cell chain hide under the next matmul phase, and h is written
# by the cell directly into the bf16 tile the next step's matmuls read
# (block-major, so the per-iteration DRAM stores are contiguous).
import functools
import numpy as np
import ml_dtypes

B, T, A, M = 32, 1000, 512, 80
P, H = 256, 1024
NCORES = 8
BC = B // NCORES            # 4 sequences per core
F = BC * T                  # 4000 frames per core, frame f = t*BC + b
G4 = 4 * H                  # 4096 gate rows
NBLK = H // 128             # 8 channel blocks
SBLK = 20                   # recurrence steps per hardware-loop iteration
# gate order used on-chip: i, f, o, g  (PyTorch order is i, f, g, o)
GORDER = (0, 1, 3, 2)
NCHUNK = 8                  # frame chunks for batched GEMMs
FCH = F // NCHUNK           # 500 frames per chunk


def _arrange_cols(wt):
    """wt [K, 4096] (= w.T, PyTorch gate order i,f,g,o on columns) ->
    m-tile order: half(2: blocks 0-3 / 4-7) x gi(4: i,f,o,g) x blk(4)."""
    cols = []
    for half in range(2):
        for go in GORDER:
            for bl in range(4):
                blk = half * 4 + bl
                cols.append(wt[:, go * H + blk * 128: go * H + (blk + 1) * 128])
    return np.ascontiguousarray(np.concatenate(cols, axis=1))


def _arrange_vec(b):
    return _arrange_cols(b.reshape(1, G4))[0]


@functools.lru_cache(maxsize=1)
def _build():
    import concourse.bacc as bacc
    import concourse.mybir as mybir
    from concourse import tile
    import concourse.bass as bass

    dt = mybir.dt
    nc = bacc.Bacc(None)

    # ---------------- I/O (host supplies channel-major tensors) ----------------
    memT = nc.declare_dram_parameter("memT", [A, F], dt.float32, isOutput=False)
    prevTin = nc.declare_dram_parameter("prevTin", [M, F], dt.float32, isOutput=False)
    w1t = nc.declare_dram_parameter("w1t", [M, P], dt.float32, isOutput=False)
    w2t = nc.declare_dram_parameter("w2t", [P, P], dt.float32, isOutput=False)
    wih0t = nc.declare_dram_parameter("wih0t", [P + A, G4], dt.float32, isOutput=False)
    whh0t = nc.declare_dram_parameter("whh0t", [H, G4], dt.bfloat16, isOutput=False)
    wih1t = nc.declare_dram_parameter("wih1t", [H, G4], dt.bfloat16, isOutput=False)
    whh1t = nc.declare_dram_parameter("whh1t", [H, G4], dt.bfloat16, isOutput=False)
    b0in = nc.declare_dram_parameter("b0in", [1, G4], dt.float32, isOutput=False)
    b1in = nc.declare_dram_parameter("b1in", [1, G4], dt.float32, isOutput=False)
    wpt_h = nc.declare_dram_parameter("wpt_h", [H, M], dt.bfloat16, isOutput=False)
    wpt_m = nc.declare_dram_parameter("wpt_m", [A, M], dt.float32, isOutput=False)
    bpin = nc.declare_dram_parameter("bpin", [1, M], dt.float32, isOutput=False)
    outT = nc.declare_dram_parameter("outT", [M, F], dt.float32, isOutput=True)

    # ---------------- internal DRAM ----------------
    xg0T = nc.dram_tensor("xg0T", [G4, F], dt.bfloat16)
    h0T = nc.dram_tensor("h0T", [H, F], dt.bfloat16)
    xg1T = nc.dram_tensor("xg1T", [G4, F], dt.bfloat16)
    h1T = nc.dram_tensor("h1T", [H, F], dt.bfloat16)

    ACT = mybir.ActivationFunctionType

    with tile.TileContext(nc) as tc:
        with tc.tile_pool(name="const", bufs=1) as cpool:
            b0sb = cpool.tile([128, 32], dt.float32, name="b0sb")
            b1sb = cpool.tile([128, 32], dt.float32, name="b1sb")
            bpsb = cpool.tile([M, 1], dt.float32, name="bpsb")
            # bias column m at b*sb[:, m]
            nc.sync.dma_start(b0sb[:], b0in[:].rearrange("o (m p) -> (o p) m", p=128))
            nc.sync.dma_start(b1sb[:], b1in[:].rearrange("o (m p) -> (o p) m", p=128))
            nc.sync.dma_start(bpsb[:], bpin[:].rearrange("o (m u) -> (o m) u", u=1))

            # persistent channel-major activations
            with tc.tile_pool(name="actsb", bufs=1) as apool:
                p2T = apool.tile([128, 2 * F], dt.float32, name="p2T")

                # ---------- Ph2: prenet ----------
                with tc.tile_pool(name="pn", bufs=2) as pnp, \
                     tc.tile_pool(name="pnps", bufs=2, space="PSUM") as pnps:
                    prevT = pnp.tile([M, F], dt.float32, name="prevT")
                    nc.sync.dma_start(prevT[:], prevTin[:])
                    w1sb = pnp.tile([M, P], dt.float32, name="w1sb")
                    nc.sync.dma_start(w1sb[:], w1t[:])
                    p1T = pnp.tile([128, 2 * F], dt.float32, name="p1T")
                    for m in range(P // 128):
                        for n in range(NCHUNK):
                            ps = pnps.tile([128, FCH], dt.float32, name="pnps1", tag=f"pn{n % 4}")
                            nc.tensor.matmul(ps[:], w1sb[:, m * 128:(m + 1) * 128],
                                             prevT[:, n * FCH:(n + 1) * FCH], start=True, stop=True)
                            nc.scalar.activation(p1T[:, m * F + n * FCH: m * F + (n + 1) * FCH], ps[:], ACT.Relu)
                    w2sb = pnp.tile([128, 2 * P], dt.float32, name="w2sb")
                    for k in range(P // 128):
                        nc.sync.dma_start(w2sb[:, k * P:(k + 1) * P], w2t[k * 128:(k + 1) * 128, :])
                    for m in range(P // 128):
                        for n in range(NCHUNK):
                            ps = pnps.tile([128, FCH], dt.float32, name="pnps2", tag=f"pn{n % 4}")
                            for k in range(P // 128):
                                nc.tensor.matmul(ps[:], w2sb[:, k * P + m * 128: k * P + (m + 1) * 128],
                                                 p1T[:, k * F + n * FCH: k * F + (n + 1) * FCH],
                                                 start=(k == 0), stop=(k == 1))
                            nc.scalar.activation(p2T[:, m * F + n * FCH: m * F + (n + 1) * FCH], ps[:], ACT.Relu)

                # ---------- Ph3: xg0 ----------
                # rhs K-tiles: 2 from p2T, 4 from memT (SBUF-resident copy)
                KX = 6
                with tc.tile_pool(name="x0", bufs=2) as x0p, \
                     tc.tile_pool(name="x0ps", bufs=1, space="PSUM") as x0ps:
                    memTsb = x0p.tile([128, 4 * F], dt.float32, name="memTsb")
                    for cb in range(4):
                        nc.sync.dma_start(memTsb[:, cb * F:(cb + 1) * F], memT[cb * 128:(cb + 1) * 128, :])

                    def x_rhs(k, n):
                        if k < 2:
                            return p2T[:, k * F + n * FCH: k * F + (n + 1) * FCH]
                        cb = k - 2
                        return memTsb[:, cb * F + n * FCH: cb * F + n * FCH + FCH]

                    for m in range(32):
                        wtile = x0p.tile([128, 6 * 128], dt.float32, name="wtile", tag="w0t")
                        for k in range(KX):
                            nc.sync.dma_start(
                                wtile[:, k * 128:(k + 1) * 128],
                                wih0t[k * 128:(k + 1) * 128, m * 128:(m + 1) * 128])
                        pss = []
                        for n in range(NCHUNK):
                            ps = x0ps.tile([128, FCH], dt.float32, name="x0psn", tag=f"x0{n}")
                            pss.append(ps)
                        for k in range(KX):
                            for n in range(NCHUNK):
                                nc.tensor.matmul(pss[n][:], wtile[:, k * 128:(k + 1) * 128], x_rhs(k, n),
                                                 start=(k == 0), stop=(k == KX - 1))
                        for n in range(NCHUNK):
                            otile = x0p.tile([128, FCH], dt.bfloat16, name="otile", tag="x0o")
                            nc.vector.tensor_scalar_add(otile[:], pss[n][:], b0sb[:, m:m + 1])
                            nc.sync.dma_start(xg0T[m * 128:(m + 1) * 128, n * FCH:(n + 1) * FCH], otile[:])

            # ---------- recurrence helper ----------
            def recurrence(whhT_in, xgT_d, hT_out):
                NB = T // SBLK
                S4 = SBLK * 4
                with tc.tile_pool(name="rc", bufs=1) as rp, \
                     tc.tile_pool(name="rcx", bufs=2) as rxp, \
                     tc.tile_pool(name="rcps", bufs=1, space="PSUM") as rps, \
                     tc.tile_pool(name="rct", bufs=2) as rtp:
                    whsb = rp.tile([128, 8 * G4], dt.bfloat16, name="whsb")
                    for k in range(8):
                        nc.sync.dma_start(whsb[:, k * G4:(k + 1) * G4], whhT_in[k * 128:(k + 1) * 128, :])

                    def wh(k, mm):
                        return whsb[:, k * G4 + mm * 128: k * G4 + (mm + 1) * 128]

                    # h per half, block-major: col blkloc*S4 + s*4 + b
                    hA = rp.tile([128, 4 * S4], dt.bfloat16, name="hA")
                    hB = rp.tile([128, 4 * S4], dt.bfloat16, name="hB")
                    cA = [rp.tile([128, 16], dt.float32, name=f"cA{i}") for i in range(2)]
                    cB = [rp.tile([128, 16], dt.float32, name=f"cB{i}") for i in range(2)]
                    nc.gpsimd.memset(hA[:], 0.0)
                    nc.gpsimd.memset(hB[:], 0.0)
                    for i in range(2):
                        nc.gpsimd.memset(cA[i][:], 0.0)
                        nc.gpsimd.memset(cB[i][:], 0.0)
                    # psum: full banks; psLo holds k0-3 partials for all 32 mm,
                    # psHiA/B hold k4-7 partials for mm 0-15 / 16-31
                    psLo = [rps.tile([128, 512], dt.float32, name=f"psLo{i}") for i in range(2)]
                    psHiA = [rps.tile([128, 512], dt.float32, name=f"psHiA{i}") for i in range(2)]
                    psHiB = [rps.tile([128, 512], dt.float32, name=f"psHiB{i}") for i in range(2)]

                    def rhs(ht, kloc, pv):
                        return ht[:, kloc * S4 + pv * 4: kloc * S4 + pv * 4 + 4]

                    def cell_pre(zt, psl, xg3):
                        # zt = psLo-part + xg   (off critical path)
                        z3 = zt[:].rearrange("p (m b) -> p m b", m=16)
                        nc.vector.tensor_add(z3, psl.rearrange("p (m b) -> p m b", m=16), xg3)

                    def cell_main(zt, psh, c_in, c_out, h_t, s, tagc):
                        # zt += psHi; gates: cols gi*16+blk*4+b, gi in (i,f,o,g)
                        nc.vector.tensor_add(zt[:], zt[:], psh)
                        st = rtp.tile([128, 48], dt.float32, name="st", tag=f"st{tagc}")
                        nc.scalar.activation(st[:], zt[:, 0:48], ACT.Sigmoid)
                        gt = rtp.tile([128, 16], dt.float32, name="gt", tag=f"gt{tagc}")
                        nc.scalar.activation(gt[:], zt[:, 48:64], ACT.Tanh)
                        aa = rtp.tile([128, 16], dt.float32, name="aa", tag=f"aa{tagc}")
                        nc.vector.tensor_mul(aa[:], st[:, 16:32], c_in[:])
                        bb = rtp.tile([128, 16], dt.float32, name="bb", tag=f"bb{tagc}")
                        nc.vector.tensor_mul(bb[:], st[:, 0:16], gt[:])
                        nc.vector.tensor_add(c_out[:], aa[:], bb[:])
                        tcx = rtp.tile([128, 16], dt.float32, name="tcx", tag=f"tc{tagc}")
                        nc.scalar.activation(tcx[:], c_out[:], ACT.Tanh)
                        # h (bf16) into block-major slot s
                        h3 = h_t[:].rearrange("p (blk sb) -> p blk sb", blk=4)[:, :, s * 4:(s + 1) * 4]
                        o3 = st[:, 32:48].rearrange("p (blk b) -> p blk b", blk=4)
                        t3 = tcx[:].rearrange("p (blk b) -> p blk b", blk=4)
                        nc.vector.tensor_mul(h3, o3, t3)

                    with tc.For_i(0, NB, 1, hint_engines=(mybir.EngineType.PE,
                                                          mybir.EngineType.DVE,
                                                          mybir.EngineType.Activation)) as bi:
                        # xg for SBLK steps: col rr*S4 + s*4 + b  (rr = m-tile)
                        xgsb = rxp.tile([128, 32 * S4], dt.bfloat16, name="xgsb", tag="xgsb")
                        for rr in range(32):
                            nc.sync.dma_start(
                                xgsb[:, rr * S4:(rr + 1) * S4],
                                xgT_d[rr * 128:(rr + 1) * 128, bass.ts(bi, S4)])
                        xgv = xgsb[:].rearrange("p (rr sb) -> p rr sb", rr=32)

                        for s in range(SBLK):
                            q = s % 2
                            pv = (s - 1) % SBLK
                            # P1: all mm, k0-3 -> psLo (contiguous 4-mm groups)
                            for mm in range(32):
                                dst = psLo[q][:, mm * 4: mm * 4 + 4]
                                for k in range(4):
                                    nc.tensor.matmul(dst, wh(k, mm), rhs(hA, k, pv),
                                                     start=(k == 0), stop=(k == 3))
                                if mm == 15:
                                    ztA = rtp.tile([128, 64], dt.float32, name="ztA", tag=f"zt{q}A")
                                    cell_pre(ztA, psLo[q][:, 0:64], xgv[:, 0:16, s * 4:(s + 1) * 4])
                            ztB = rtp.tile([128, 64], dt.float32, name="ztB", tag=f"zt{q}B")
                            cell_pre(ztB, psLo[q][:, 64:128], xgv[:, 16:32, s * 4:(s + 1) * 4])
                            # P2: mm 0-15, k4-7 -> psHiA
                            for mm in range(16):
                                dst = psHiA[q][:, mm * 4: mm * 4 + 4]
                                for k in range(4, 8):
                                    nc.tensor.matmul(dst, wh(k, mm), rhs(hB, k - 4, pv),
                                                     start=(k == 4), stop=(k == 7))
                            cell_main(ztA, psHiA[q][:, 0:64], cA[1 - q], cA[q],
                                      hA, s, f"{q}A")
                            # P3: mm 16-31, k4-7 -> psHiB
                            for mm in range(16, 32):
                                dst = psHiB[q][:, (mm - 16) * 4:(mm - 16) * 4 + 4]
                                for k in range(4, 8):
                                    nc.tensor.matmul(dst, wh(k, mm), rhs(hB, k - 4, pv),
                                                     start=(k == 4), stop=(k == 7))
                            cell_main(ztB, psHiB[q][:, 0:64], cB[1 - q], cB[q],
                                      hB, s, f"{q}B")

                        # contiguous h stores (scalar queue, so sync-queue xg
                        # loads for the next iteration are not blocked)
                        for half, ht in ((0, hA), (1, hB)):
                            for bl in range(4):
                                gb = half * 4 + bl
                                nc.scalar.dma_start(
                                    hT_out[gb * 128:(gb + 1) * 128, bass.ts(bi, S4)],
                                    ht[:, bl * S4:(bl + 1) * S4])

            # ---------- Ph4: layer-0 recurrence ----------
            recurrence(whh0t, xg0T, h0T)

            # ---------- Ph5: xg1 ----------
            with tc.tile_pool(name="x1", bufs=1) as x1p, \
                 tc.tile_pool(name="x1w", bufs=2) as x1wp, \
                 tc.tile_pool(name="x1ps", bufs=1, space="PSUM") as x1ps:
                h0sb = x1p.tile([128, 8 * F], dt.bfloat16, name="h0sb")
                for k in range(8):
                    nc.sync.dma_start(h0sb[:, k * F:(k + 1) * F], h0T[k * 128:(k + 1) * 128, :])
                for m in range(32):
                    wtile = x1wp.tile([128, 8 * 128], dt.bfloat16, name="w1tile", tag="w1t")
                    for k in range(8):
                        nc.sync.dma_start(
                            wtile[:, k * 128:(k + 1) * 128],
                            wih1t[k * 128:(k + 1) * 128, m * 128:(m + 1) * 128])
                    pss = []
                    for n in range(NCHUNK):
                        ps = x1ps.tile([128, FCH], dt.float32, name="x1psn", tag=f"x1{n}")
                        pss.append(ps)
                    for k in range(8):
                        for n in range(NCHUNK):
                            nc.tensor.matmul(pss[n][:], wtile[:, k * 128:(k + 1) * 128],
                                             h0sb[:, k * F + n * FCH: k * F + n * FCH + FCH],
                                             start=(k == 0), stop=(k == 7))
                    for n in range(NCHUNK):
                        otile = x1wp.tile([128, FCH], dt.bfloat16, name="o1tile", tag="x1o")
                        nc.vector.tensor_scalar_add(otile[:], pss[n][:], b1sb[:, m:m + 1])
                        nc.sync.dma_start(xg1T[m * 128:(m + 1) * 128, n * FCH:(n + 1) * FCH], otile[:])

            # ---------- Ph6: layer-1 recurrence ----------
            recurrence(whh1t, xg1T, h1T)

            # ---------- Ph7: projection ----------
            with tc.tile_pool(name="pj", bufs=1) as pjp, \
                 tc.tile_pool(name="pjw", bufs=2) as pjwp, \
                 tc.tile_pool(name="pjps", bufs=2, space="PSUM") as pjps:
                h1sb = pjp.tile([128, 8 * F], dt.bfloat16, name="h1sb")
                for k in range(8):
                    nc.sync.dma_start(h1sb[:, k * F:(k + 1) * F], h1T[k * 128:(k + 1) * 128, :])
                memTsb2 = pjp.tile([128, 4 * F], dt.float32, name="memTsb2")
                for cb in range(4):
                    nc.sync.dma_start(memTsb2[:, cb * F:(cb + 1) * F], memT[cb * 128:(cb + 1) * 128, :])
                wphsb = pjp.tile([128, 8 * M], dt.bfloat16, name="wphsb")
                for k in range(8):
                    nc.sync.dma_start(wphsb[:, k * M:(k + 1) * M], wpt_h[k * 128:(k + 1) * 128, :])
                wpmsb = pjp.tile([128, 4 * M], dt.float32, name="wpmsb")
                for k in range(4):
                    nc.sync.dma_start(wpmsb[:, k * M:(k + 1) * M], wpt_m[k * 128:(k + 1) * 128, :])
                for n in range(NCHUNK):
                    ps = pjps.tile([M, FCH], dt.float32, name="pjpsn", tag=f"pj{n % 4}")
                    for k in range(8):
                        nc.tensor.matmul(ps[:], wphsb[:, k * M:(k + 1) * M],
                                         h1sb[:, k * F + n * FCH: k * F + n * FCH + FCH],
                                         start=(k == 0), stop=False)
                    for cb in range(4):
                        nc.tensor.matmul(ps[:], wpmsb[:, cb * M:(cb + 1) * M],
                                         memTsb2[:, cb * F + n * FCH: cb * F + n * FCH + FCH],
                                         start=False, stop=(cb == 3))
                    otile = pjwp.tile([M, FCH], dt.float32, name="pjo", tag="pjo")
                    nc.vector.tensor_scalar_add(otile[:], ps[:], bpsb[:, 0:1])
                    nc.sync.dma_start(outT[:, n * FCH:(n + 1) * FCH], otile[:])

    nc.finalize()
    return nc


def kernel(memory, y_mels, W1, W2, w_ih0, w_hh0, b_ih0, b_hh0,
           w_ih1, w_hh1, b_ih1, b_hh1, W_proj, b_proj, _trace=False):
    from concourse.bass_utils import run_bass_kernel_spmd

    nc = _build()
    bf16 = ml_dtypes.bfloat16
    w1t = np.ascontiguousarray(W1.T.astype(np.float32))
    w2t = np.ascontiguousarray(W2.T.astype(np.float32))
    wih0t = _arrange_cols(w_ih0.T.astype(np.float32))
    whh0t = _arrange_cols(w_hh0.T.astype(np.float32)).astype(bf16)
    wih1t = _arrange_cols(w_ih1.T.astype(np.float32)).astype(bf16)
    whh1t = _arrange_cols(w_hh1.T.astype(np.float32)).astype(bf16)
    b0 = _arrange_vec((b_ih0 + b_hh0).astype(np.float32)).reshape(1, G4)
    b1 = _arrange_vec((b_ih1 + b_hh1).astype(np.float32)).reshape(1, G4)
    wpt = W_proj.T.astype(np.float32)
    wpt_h = np.ascontiguousarray(wpt[:H]).astype(bf16)
    wpt_m = np.ascontiguousarray(wpt[H:])
    bp = b_proj.astype(np.float32).reshape(1, M)

    in_maps = []
    for c in range(NCORES):
        mem_c = memory[c * BC:(c + 1) * BC]          # [4, 1000, 512]
        y_c = y_mels[c * BC:(c + 1) * BC]            # [4, 1000, 80]
        # channel-major [A, F] / shifted mels [M, F], frame f = t*BC + b
        memT_c = np.ascontiguousarray(
            mem_c.transpose(2, 1, 0).reshape(A, F).astype(np.float32))
        prev_c = np.concatenate(
            [np.zeros((BC, 1, M), np.float32), y_c[:, :-1, :]], axis=1)
        prevT_c = np.ascontiguousarray(
            prev_c.transpose(2, 1, 0).reshape(M, F).astype(np.float32))
        in_maps.append(dict(
            memT=memT_c, prevTin=prevT_c, w1t=w1t, w2t=w2t,
            wih0t=wih0t, whh0t=whh0t, wih1t=wih1t, whh1t=whh1t,
            b0in=b0, b1in=b1, wpt_h=wpt_h, wpt_m=wpt_m, bpin=bp))

    res = run_bass_kernel_spmd(nc, in_maps, core_ids=list(range(NCORES)), trace=_trace)
    outs = []
    for c in range(NCORES):
        oT = res.results[c]["outT"]                  # [80, 4000]
        outs.append(oT.reshape(M, T, BC).transpose(2, 1, 0))  # [4, 1000, 80]
    full = np.concatenate(outs, axis=0).astype(np.float32)
    if _trace:
        kernel.last_exec_time_ns = res.exec_time_ns
    return full


# revision 4
# speedup vs baseline: 1.6367x; 1.0698x over previous
# Trainium2 Bass kernel for the Tacotron-style decoder (2-layer LSTM, B=32,
# T=1000). Strategy: data-parallel over batch, 4 sequences per NeuronCore.
# All compute is local per core (no collectives).  Host pre-transposes the
# inputs to channel-major, so the device phases are:
#   Ph2  prenet (2x matmul+relu), channel-major
#   Ph3  xg0 = w_ih0 @ x + b   (batched over all timesteps)
#   Ph4  layer-0 LSTM recurrence
#   Ph5  xg1 = w_ih1 @ h0 + b  (batched)
#   Ph6  layer-1 LSTM recurrence
#   Ph7  projection out = W_proj @ [h1; mem] + b
#
# Recurrence (the hot 92%): per step, 256 LDWEIGHTS+MATMUL pairs (32 gate
# m-tiles x 8 k-tiles, N=4 batch).  Gate m-tiles are ordered gi-major per
# half (half A = channel blocks 0-3, B = 4-7; within a half: i,f,o,g x blk)
# so every LSTM-cell op is a contiguous 2D tile op.  Accumulation is split
# into psLo (k0-3) / psHi (k4-7) per-mm contiguous groups; schedule
#   P1: all mm kLo | P2: mmA kHi | cellA | P3: mmB kHi | cellB
# lets each cell chain hide under the next matmul phase, and h is written
# by the cell directly into the bf16 tile the next step's matmuls read
# (block-major, so the per-iteration DRAM stores are contiguous).
import functools
import numpy as np
import ml_dtypes

B, T, A, M = 32, 1000, 512, 80
P, H = 256, 1024
NCORES = 8
BC = B // NCORES            # 4 sequences per core
F = BC * T                  # 4000 frames per core, frame f = t*BC + b
G4 = 4 * H                  # 4096 gate rows
NBLK = H // 128             # 8 channel blocks
SBLK = 20                   # recurrence steps per hardware-loop iteration
# gate order used on-chip: i, f, o, g  (PyTorch order is i, f, g, o)
GORDER = (0, 1, 3, 2)
NCHUNK = 8                  # frame chunks for batched GEMMs
FCH = F // NCHUNK           # 500 frames per chunk


def _arrange_cols(wt):
    """wt [K, 4096] (= w.T, PyTorch gate order i,f,g,o on columns) ->
    m-tile order: half(2: blocks 0-3 / 4-7) x gi(4: i,f,o,g) x blk(4)."""
    cols = []
    for half in range(2):
        for go in GORDER:
            for bl in range(4):
                blk = half * 4 + bl
                cols.append(wt[:, go * H + blk * 128: go * H + (blk + 1) * 128])
    return np.ascontiguousarray(np.concatenate(cols, axis=1))


def _arrange_vec(b):
    return _arrange_cols(b.reshape(1, G4))[0]


@functools.lru_cache(maxsize=1)
def _build():
    import concourse.bacc as bacc
    import concourse.mybir as mybir
    from concourse import tile
    import concourse.bass as bass

    dt = mybir.dt
    nc = bacc.Bacc(None)

    # ---------------- I/O (host supplies channel-major tensors) ----------------
    memT = nc.declare_dram_parameter("memT", [A, F], dt.float32, isOutput=False)
    prevTin = nc.declare_dram_parameter("prevTin", [M, F], dt.float32, isOutput=False)
    w1t = nc.declare_dram_parameter("w1t", [M, P], dt.float32, isOutput=False)
    w2t = nc.declare_dram_parameter("w2t", [P, P], dt.float32, isOutput=False)
    wih0t = nc.declare_dram_parameter("wih0t", [P + A, G4], dt.float32, isOutput=False)
    whh0t = nc.declare_dram_parameter("whh0t", [H, G4], dt.bfloat16, isOutput=False)
    wih1t = nc.declare_dram_parameter("wih1t", [H, G4], dt.bfloat16, isOutput=False)
    whh1t = nc.declare_dram_parameter("whh1t", [H, G4], dt.bfloat16, isOutput=False)
    b0in = nc.declare_dram_parameter("b0in", [1, G4], dt.float32, isOutput=False)
    b1in = nc.declare_dram_parameter("b1in", [1, G4], dt.float32, isOutput=False)
    wpt_h = nc.declare_dram_parameter("wpt_h", [H, M], dt.bfloat16, isOutput=False)
    wpt_m = nc.declare_dram_parameter("wpt_m", [A, M], dt.float32, isOutput=False)
    bpin = nc.declare_dram_parameter("bpin", [1, M], dt.float32, isOutput=False)
    outT = nc.declare_dram_parameter("outT", [M, F], dt.float32, isOutput=True)

    # ---------------- internal DRAM ----------------
    xg0T = nc.dram_tensor("xg0T", [G4, F], dt.bfloat16)
    h0T = nc.dram_tensor("h0T", [H, F], dt.bfloat16)
    xg1T = nc.dram_tensor("xg1T", [G4, F], dt.bfloat16)
    h1T = nc.dram_tensor("h1T", [H, F], dt.bfloat16)

    ACT = mybir.ActivationFunctionType

    with tile.TileContext(nc) as tc:
        with tc.tile_pool(name="const", bufs=1) as cpool:
            b0sb = cpool.tile([128, 32], dt.float32, name="b0sb")
            b1sb = cpool.tile([128, 32], dt.float32, name="b1sb")
            bpsb = cpool.tile([M, 1], dt.float32, name="bpsb")
            # bias column m at b*sb[:, m]
            nc.sync.dma_start(b0sb[:], b0in[:].rearrange("o (m p) -> (o p) m", p=128))
            nc.sync.dma_start(b1sb[:], b1in[:].rearrange("o (m p) -> (o p) m", p=128))
            nc.sync.dma_start(bpsb[:], bpin[:].rearrange("o (m u) -> (o m) u", u=1))

            # persistent channel-major activations
            with tc.tile_pool(name="actsb", bufs=1) as apool:
                p2T = apool.tile([128, 2 * F], dt.float32, name="p2T")

                # ---------- Ph2: prenet ----------
                with tc.tile_pool(name="pn", bufs=2) as pnp, \
                     tc.tile_pool(name="pnps", bufs=2, space="PSUM") as pnps:
                    prevT = pnp.tile([M, F], dt.float32, name="prevT")
                    nc.sync.dma_start(prevT[:], prevTin[:])
                    w1sb = pnp.tile([M, P], dt.float32, name="w1sb")
                    nc.sync.dma_start(w1sb[:], w1t[:])
                    p1T = pnp.tile([128, 2 * F], dt.float32, name="p1T")
                    for m in range(P // 128):
                        for n in range(NCHUNK):
                            ps = pnps.tile([128, FCH], dt.float32, name="pnps1", tag=f"pn{n % 4}")
                            nc.tensor.matmul(ps[:], w1sb[:, m * 128:(m + 1) * 128],
                                             prevT[:, n * FCH:(n + 1) * FCH], start=True, stop=True)
                            nc.scalar.activation(p1T[:, m * F + n * FCH: m * F + (n + 1) * FCH], ps[:], ACT.Relu)
                    w2sb = pnp.tile([128, 2 * P], dt.float32, name="w2sb")
                    for k in range(P // 128):
                        nc.sync.dma_start(w2sb[:, k * P:(k + 1) * P], w2t[k * 128:(k + 1) * 128, :])
                    for m in range(P // 128):
                        for n in range(NCHUNK):
                            ps = pnps.tile([128, FCH], dt.float32, name="pnps2", tag=f"pn{n % 4}")
                            for k in range(P // 128):
                                nc.tensor.matmul(ps[:], w2sb[:, k * P + m * 128: k * P + (m + 1) * 128],
                                                 p1T[:, k * F + n * FCH: k * F + (n + 1) * FCH],
                                                 start=(k == 0), stop=(k == 1))
                            nc.scalar.activation(p2T[:, m * F + n * FCH: m * F + (n + 1) * FCH], ps[:], ACT.Relu)

                # ---------- Ph3: xg0 ----------
                # rhs K-tiles: 2 from p2T, 4 from memT (SBUF-resident copy)
                KX = 6
                with tc.tile_pool(name="x0", bufs=2) as x0p, \
                     tc.tile_pool(name="x0ps", bufs=1, space="PSUM") as x0ps:
                    memTsb = x0p.tile([128, 4 * F], dt.float32, name="memTsb")
                    for cb in range(4):
                        nc.sync.dma_start(memTsb[:, cb * F:(cb + 1) * F], memT[cb * 128:(cb + 1) * 128, :])

                    def x_rhs(k, n):
                        if k < 2:
                            return p2T[:, k * F + n * FCH: k * F + (n + 1) * FCH]
                        cb = k - 2
                        return memTsb[:, cb * F + n * FCH: cb * F + n * FCH + FCH]

                    for m in range(32):
                        wtile = x0p.tile([128, 6 * 128], dt.float32, name="wtile", tag="w0t")
                        for k in range(KX):
                            nc.sync.dma_start(
                                wtile[:, k * 128:(k + 1) * 128],
                                wih0t[k * 128:(k + 1) * 128, m * 128:(m + 1) * 128])
                        pss = []
                        for n in range(NCHUNK):
                            ps = x0ps.tile([128, FCH], dt.float32, name="x0psn", tag=f"x0{n}")
                            pss.append(ps)
                        for k in range(KX):
                            for n in range(NCHUNK):
                                nc.tensor.matmul(pss[n][:], wtile[:, k * 128:(k + 1) * 128], x_rhs(k, n),
                                                 start=(k == 0), stop=(k == KX - 1))
                        for n in range(NCHUNK):
                            otile = x0p.tile([128, FCH], dt.bfloat16, name="otile", tag="x0o")
                            nc.vector.tensor_scalar_add(otile[:], pss[n][:], b0sb[:, m:m + 1])
                            nc.sync.dma_start(xg0T[m * 128:(m + 1) * 128, n * FCH:(n + 1) * FCH], otile[:])

            # ---------- recurrence helper ----------
            def recurrence(whhT_in, xgT_d, hT_out):
                SUB = 2                  # sub-blocks per loop body
                BODY = SUB * SBLK        # steps per loop body
                NB = T // BODY
                S4 = SBLK * 4
                with tc.tile_pool(name="rc", bufs=1) as rp, \
                     tc.tile_pool(name="rcx", bufs=2) as rxp, \
                     tc.tile_pool(name="rcps", bufs=1, space="PSUM") as rps, \
                     tc.tile_pool(name="rct", bufs=2) as rtp:
                    whsb = rp.tile([128, 8 * G4], dt.bfloat16, name="whsb")
                    for k in range(8):
                        nc.sync.dma_start(whsb[:, k * G4:(k + 1) * G4], whhT_in[k * 128:(k + 1) * 128, :])

                    def wh(k, mm):
                        return whsb[:, k * G4 + mm * 128: k * G4 + (mm + 1) * 128]

                    # h per half and sub-block, block-major: col blkloc*S4 + s*4 + b
                    hA = [rp.tile([128, 4 * S4], dt.bfloat16, name=f"hA{u}") for u in range(SUB)]
                    hB = [rp.tile([128, 4 * S4], dt.bfloat16, name=f"hB{u}") for u in range(SUB)]
                    cA = [rp.tile([128, 16], dt.float32, name=f"cA{i}") for i in range(2)]
                    cB = [rp.tile([128, 16], dt.float32, name=f"cB{i}") for i in range(2)]
                    for u in range(SUB):
                        nc.gpsimd.memset(hA[u][:], 0.0)
                        nc.gpsimd.memset(hB[u][:], 0.0)
                    for i in range(2):
                        nc.gpsimd.memset(cA[i][:], 0.0)
                        nc.gpsimd.memset(cB[i][:], 0.0)
                    # psum: full banks; psLo holds k0-3 partials for all 32 mm,
                    # psHiA/B hold k4-7 partials for mm 0-15 / 16-31
                    psLo = [rps.tile([128, 512], dt.float32, name=f"psLo{i}") for i in range(2)]
                    psHiA = [rps.tile([128, 512], dt.float32, name=f"psHiA{i}") for i in range(2)]
                    psHiB = [rps.tile([128, 512], dt.float32, name=f"psHiB{i}") for i in range(2)]

                    def rhs(hlist, g, kloc):
                        # h written at global step g (of BODY), read as rhs
                        u, sl = (g // SBLK) % SUB, g % SBLK
                        ht = hlist[u]
                        return ht[:, kloc * S4 + sl * 4: kloc * S4 + sl * 4 + 4]

                    def cell_pre(zt, psl, xg3):
                        # zt = psLo-part + xg   (off critical path)
                        z3 = zt[:].rearrange("p (m b) -> p m b", m=16)
                        nc.vector.tensor_add(z3, psl.rearrange("p (m b) -> p m b", m=16), xg3)

                    def cell_main(zt, psh, c_in, c_out, h_t, sl, tagc):
                        # zt += psHi; gates: cols gi*16+blk*4+b, gi in (i,f,o,g)
                        nc.vector.tensor_add(zt[:], zt[:], psh)
                        st = rtp.tile([128, 48], dt.float32, name="st", tag=f"st{tagc}")
                        nc.scalar.activation(st[:], zt[:, 0:48], ACT.Sigmoid)
                        gt = rtp.tile([128, 16], dt.float32, name="gt", tag=f"gt{tagc}")
                        nc.scalar.activation(gt[:], zt[:, 48:64], ACT.Tanh)
                        aa = rtp.tile([128, 16], dt.float32, name="aa", tag=f"aa{tagc}")
                        nc.vector.tensor_mul(aa[:], st[:, 16:32], c_in[:])
                        bb = rtp.tile([128, 16], dt.float32, name="bb", tag=f"bb{tagc}")
                        nc.vector.tensor_mul(bb[:], st[:, 0:16], gt[:])
                        nc.vector.tensor_add(c_out[:], aa[:], bb[:])
                        tcx = rtp.tile([128, 16], dt.float32, name="tcx", tag=f"tc{tagc}")
                        nc.scalar.activation(tcx[:], c_out[:], ACT.Tanh)
                        # h (bf16) into block-major slot sl
                        h3 = h_t[:].rearrange("p (blk sb) -> p blk sb", blk=4)[:, :, sl * 4:(sl + 1) * 4]
                        o3 = st[:, 32:48].rearrange("p (blk b) -> p blk b", blk=4)
                        t3 = tcx[:].rearrange("p (blk b) -> p blk b", blk=4)
                        nc.vector.tensor_mul(h3, o3, t3)

                    with tc.For_i(0, NB, 1, staggered_reset=True,
                                  hint_engines=(mybir.EngineType.PE,
                                                mybir.EngineType.DVE,
                                                mybir.EngineType.Activation)) as bi:
                        xgt = []
                        for u in range(SUB):
                            # xg cols rr*S4 + s*4 + b  (rr = m-tile)
                            xgsb = rxp.tile([128, 32 * S4], dt.bfloat16, name="xgsb", tag=f"xg{u}")
                            xgw = xgT_d[:, bass.ts(bi, SUB * S4)]
                            for rr in range(32):
                                nc.sync.dma_start(
                                    xgsb[:, rr * S4:(rr + 1) * S4],
                                    xgw[rr * 128:(rr + 1) * 128, u * S4:(u + 1) * S4])
                            xgt.append(xgsb[:].rearrange("p (rr sb) -> p rr sb", rr=32))

                        for g in range(BODY):
                            u, s = g // SBLK, g % SBLK
                            q = g % 2
                            xgv = xgt[u]
                            gp = (g - 1) % BODY
                            # P1: all mm, k0-3 -> psLo (contiguous 4-mm groups)
                            for mm in range(32):
                                dst = psLo[q][:, mm * 4: mm * 4 + 4]
                                for k in range(4):
                                    nc.tensor.matmul(dst, wh(k, mm), rhs(hA, gp, k),
                                                     start=(k == 0), stop=(k == 3))
                                if mm == 15:
                                    ztA = rtp.tile([128, 64], dt.float32, name="ztA", tag=f"zt{q}A")
                                    cell_pre(ztA, psLo[q][:, 0:64], xgv[:, 0:16, s * 4:(s + 1) * 4])
                            ztB = rtp.tile([128, 64], dt.float32, name="ztB", tag=f"zt{q}B")
                            cell_pre(ztB, psLo[q][:, 64:128], xgv[:, 16:32, s * 4:(s + 1) * 4])
                            # P2: mm 0-15, k4-7 -> psHiA
                            for mm in range(16):
                                dst = psHiA[q][:, mm * 4: mm * 4 + 4]
                                for k in range(4, 8):
                                    nc.tensor.matmul(dst, wh(k, mm), rhs(hB, gp, k - 4),
                                                     start=(k == 4), stop=(k == 7))
                            cell_main(ztA, psHiA[q][:, 0:64], cA[1 - q], cA[q],
                                      hA[u], s, f"{q}A")
                            # P3: mm 16-31, k4-7 -> psHiB
                            for mm in range(16, 32):
                                dst = psHiB[q][:, (mm - 16) * 4:(mm - 16) * 4 + 4]
                                for k in range(4, 8):
                                    nc.tensor.matmul(dst, wh(k, mm), rhs(hB, gp, k - 4),
                                                     start=(k == 4), stop=(k == 7))
                            cell_main(ztB, psHiB[q][:, 0:64], cB[1 - q], cB[q],
                                      hB[u], s, f"{q}B")

                            if s == SBLK - 1:
                                # contiguous h stores for this sub-block (scalar
                                # queue, so sync-queue xg loads are not blocked)
                                hw = hT_out[:, bass.ts(bi, SUB * S4)]
                                for half, ht in ((0, hA[u]), (1, hB[u])):
                                    for bl in range(4):
                                        gb = half * 4 + bl
                                        nc.scalar.dma_start(
                                            hw[gb * 128:(gb + 1) * 128, u * S4:(u + 1) * S4],
                                            ht[:, bl * S4:(bl + 1) * S4])

            # ---------- Ph4: layer-0 recurrence ----------
            recurrence(whh0t, xg0T, h0T)

            # ---------- Ph5: xg1 ----------
            with tc.tile_pool(name="x1", bufs=1) as x1p, \
                 tc.tile_pool(name="x1w", bufs=2) as x1wp, \
                 tc.tile_pool(name="x1ps", bufs=1, space="PSUM") as x1ps:
                h0sb = x1p.tile([128, 8 * F], dt.bfloat16, name="h0sb")
                for k in range(8):
                    nc.sync.dma_start(h0sb[:, k * F:(k + 1) * F], h0T[k * 128:(k + 1) * 128, :])
                for m in range(32):
                    wtile = x1wp.tile([128, 8 * 128], dt.bfloat16, name="w1tile", tag="w1t")
                    for k in range(8):
                        nc.sync.dma_start(
                            wtile[:, k * 128:(k + 1) * 128],
                            wih1t[k * 128:(k + 1) * 128, m * 128:(m + 1) * 128])
                    pss = []
                    for n in range(NCHUNK):
                        ps = x1ps.tile([128, FCH], dt.float32, name="x1psn", tag=f"x1{n}")
                        pss.append(ps)
                    for k in range(8):
                        for n in range(NCHUNK):
                            nc.tensor.matmul(pss[n][:], wtile[:, k * 128:(k + 1) * 128],
                                             h0sb[:, k * F + n * FCH: k * F + n * FCH + FCH],
                                             start=(k == 0), stop=(k == 7))
                    for n in range(NCHUNK):
                        otile = x1wp.tile([128, FCH], dt.bfloat16, name="o1tile", tag="x1o")
                        nc.vector.tensor_scalar_add(otile[:], pss[n][:], b1sb[:, m:m + 1])
                        nc.sync.dma_start(xg1T[m * 128:(m + 1) * 128, n * FCH:(n + 1) * FCH], otile[:])

            # ---------- Ph6: layer-1 recurrence ----------
            recurrence(whh1t, xg1T, h1T)

            # ---------- Ph7: projection ----------
            with tc.tile_pool(name="pj", bufs=1) as pjp, \
                 tc.tile_pool(name="pjw", bufs=2) as pjwp, \
                 tc.tile_pool(name="pjps", bufs=2, space="PSUM") as pjps:
                h1sb = pjp.tile([128, 8 * F], dt.bfloat16, name="h1sb")
                for k in range(8):
                    nc.sync.dma_start(h1sb[:, k * F:(k + 1) * F], h1T[k * 128:(k + 1) * 128, :])
                memTsb2 = pjp.tile([128, 4 * F], dt.float32, name="memTsb2")
                for cb in range(4):
                    nc.sync.dma_start(memTsb2[:, cb * F:(cb + 1) * F], memT[cb * 128:(cb + 1) * 128, :])
                wphsb = pjp.tile([128, 8 * M], dt.bfloat16, name="wphsb")
                for k in range(8):
                    nc.sync.dma_start(wphsb[:, k * M:(k + 1) * M], wpt_h[k * 128:(k + 1) * 128, :])
                wpmsb = pjp.tile([128, 4 * M], dt.float32, name="wpmsb")
                for k in range(4):
                    nc.sync.dma_start(wpmsb[:, k * M:(k + 1) * M], wpt_m[k * 128:(k + 1) * 128, :])
                for n in range(NCHUNK):
                    ps = pjps.tile([M, FCH], dt.float32, name="pjpsn", tag=f"pj{n % 4}")
                    for k in range(8):
                        nc.tensor.matmul(ps[:], wphsb[:, k * M:(k + 1) * M],
                                         h1sb[:, k * F + n * FCH: k * F + n * FCH + FCH],
                                         start=(k == 0), stop=False)
                    for cb in range(4):
                        nc.tensor.matmul(ps[:], wpmsb[:, cb * M:(cb + 1) * M],
                                         memTsb2[:, cb * F + n * FCH: cb * F + n * FCH + FCH],
                                         start=False, stop=(cb == 3))
                    otile = pjwp.tile([M, FCH], dt.float32, name="pjo", tag="pjo")
                    nc.vector.tensor_scalar_add(otile[:], ps[:], bpsb[:, 0:1])
                    nc.sync.dma_start(outT[:, n * FCH:(n + 1) * FCH], otile[:])

    nc.finalize()
    return nc


def kernel(memory, y_mels, W1, W2, w_ih0, w_hh0, b_ih0, b_hh0,
           w_ih1, w_hh1, b_ih1, b_hh1, W_proj, b_proj, _trace=False):
    from concourse.bass_utils import run_bass_kernel_spmd

    nc = _build()
    bf16 = ml_dtypes.bfloat16
    w1t = np.ascontiguousarray(W1.T.astype(np.float32))
    w2t = np.ascontiguousarray(W2.T.astype(np.float32))
    wih0t = _arrange_cols(w_ih0.T.astype(np.float32))
    whh0t = _arrange_cols(w_hh0.T.astype(np.float32)).astype(bf16)
    wih1t = _arrange_cols(w_ih1.T.astype(np.float32)).astype(bf16)
    whh1t = _arrange_cols(w_hh1.T.astype(np.float32)).astype(bf16)
    b0 = _arrange_vec((b_ih0 + b_hh0).astype(np.float32)).reshape(1, G4)
    b1 = _arrange_vec((b_ih1 + b_hh1).astype(np.float32)).reshape(1, G4)
    wpt = W_proj.T.astype(np.float32)
    wpt_h = np.ascontiguousarray(wpt[:H]).astype(bf16)
    wpt_m = np.ascontiguousarray(wpt[H:])
    bp = b_proj.astype(np.float32).reshape(1, M)

    in_maps = []
    for c in range(NCORES):
        mem_c = memory[c * BC:(c + 1) * BC]          # [4, 1000, 512]
        y_c = y_mels[c * BC:(c + 1) * BC]            # [4, 1000, 80]
        # channel-major [A, F] / shifted mels [M, F], frame f = t*BC + b
        memT_c = np.ascontiguousarray(
            mem_c.transpose(2, 1, 0).reshape(A, F).astype(np.float32))
        prev_c = np.concatenate(
            [np.zeros((BC, 1, M), np.float32), y_c[:, :-1, :]], axis=1)
        prevT_c = np.ascontiguousarray(
            prev_c.transpose(2, 1, 0).reshape(M, F).astype(np.float32))
        in_maps.append(dict(
            memT=memT_c, prevTin=prevT_c, w1t=w1t, w2t=w2t,
            wih0t=wih0t, whh0t=whh0t, wih1t=wih1t, whh1t=whh1t,
            b0in=b0, b1in=b1, wpt_h=wpt_h, wpt_m=wpt_m, bpin=bp))

    res = run_bass_kernel_spmd(nc, in_maps, core_ids=list(range(NCORES)), trace=_trace)
    outs = []
    for c in range(NCORES):
        oT = res.results[c]["outT"]                  # [80, 4000]
        outs.append(oT.reshape(M, T, BC).transpose(2, 1, 0))  # [4, 1000, 80]
    full = np.concatenate(outs, axis=0).astype(np.float32)
    if _trace:
        kernel.last_exec_time_ns = res.exec_time_ns
    return full


# revision 6
# speedup vs baseline: 1.7553x; 1.0725x over previous
# Trainium2 Bass kernel for the Tacotron-style decoder (2-layer LSTM, B=32,
# T=1000). Strategy: data-parallel over batch, 4 sequences per NeuronCore.
# All compute is local per core (no collectives).  Host pre-transposes the
# inputs to channel-major, so the device phases are:
#   Ph2  prenet (2x matmul+relu), channel-major
#   Ph3  xg0 = w_ih0 @ x + b   (batched over all timesteps)
#   Ph4  layer-0 LSTM recurrence
#   Ph5  xg1 = w_ih1 @ h0 + b  (batched)
#   Ph6  layer-1 LSTM recurrence
#   Ph7  projection out = W_proj @ [h1; mem] + b
#
# Recurrence (the hot 92%): per step, 256 LDWEIGHTS+MATMUL pairs (32 gate
# m-tiles x 8 k-tiles, N=4 batch).  Gate m-tiles are ordered gi-major per
# half (half A = channel blocks 0-3, B = 4-7; within a half: i,f,o,g x blk)
# so every LSTM-cell op is a contiguous 2D tile op.  Accumulation is split
# into psLo (k0-3) / psHi (k4-7) per-mm contiguous groups; schedule
#   P1: all mm kLo | P2: mmA kHi | cellA | P3: mmB kHi | cellB
# lets each cell chain hide under the next matmul phase, and h is written
# by the cell directly into the bf16 tile the next step's matmuls read
# (block-major, so the per-iteration DRAM stores are contiguous).
import functools
import numpy as np
import ml_dtypes

B, T, A, M = 32, 1000, 512, 80
P, H = 256, 1024
NCORES = 8
BC = B // NCORES            # 4 sequences per core
F = BC * T                  # 4000 frames per core, frame f = t*BC + b
G4 = 4 * H                  # 4096 gate rows
NBLK = H // 128             # 8 channel blocks
SBLK = 20                   # recurrence steps per hardware-loop iteration
# gate order used on-chip: i, f, o, g  (PyTorch order is i, f, g, o)
GORDER = (0, 1, 3, 2)
NCHUNK = 8                  # frame chunks for batched GEMMs
FCH = F // NCHUNK           # 500 frames per chunk


def _arrange_cols(wt):
    """wt [K, 4096] (= w.T, PyTorch gate order i,f,g,o on columns) ->
    m-tile order: half(2: blocks 0-3 / 4-7) x gi(4: i,f,o,g) x blk(4)."""
    cols = []
    for half in range(2):
        for go in GORDER:
            for bl in range(4):
                blk = half * 4 + bl
                cols.append(wt[:, go * H + blk * 128: go * H + (blk + 1) * 128])
    return np.ascontiguousarray(np.concatenate(cols, axis=1))


def _arrange_vec(b):
    return _arrange_cols(b.reshape(1, G4))[0]


@functools.lru_cache(maxsize=1)
def _build():
    import concourse.bacc as bacc
    import concourse.mybir as mybir
    from concourse import tile
    import concourse.bass as bass

    dt = mybir.dt
    nc = bacc.Bacc(None)

    # ---------------- I/O (host supplies channel-major tensors) ----------------
    memT = nc.declare_dram_parameter("memT", [A, F], dt.bfloat16, isOutput=False)
    prevTin = nc.declare_dram_parameter("prevTin", [M, F], dt.float32, isOutput=False)
    w1t = nc.declare_dram_parameter("w1t", [M, P], dt.float32, isOutput=False)
    w2t = nc.declare_dram_parameter("w2t", [P, P], dt.float32, isOutput=False)
    wih0t = nc.declare_dram_parameter("wih0t", [P + A, G4], dt.bfloat16, isOutput=False)
    whh0t = nc.declare_dram_parameter("whh0t", [H, G4], dt.bfloat16, isOutput=False)
    wih1t = nc.declare_dram_parameter("wih1t", [H, G4], dt.bfloat16, isOutput=False)
    whh1t = nc.declare_dram_parameter("whh1t", [H, G4], dt.bfloat16, isOutput=False)
    b0in = nc.declare_dram_parameter("b0in", [1, G4], dt.float32, isOutput=False)
    b1in = nc.declare_dram_parameter("b1in", [1, G4], dt.float32, isOutput=False)
    wpt_h = nc.declare_dram_parameter("wpt_h", [H, M], dt.bfloat16, isOutput=False)
    wpt_m = nc.declare_dram_parameter("wpt_m", [A, M], dt.bfloat16, isOutput=False)
    bpin = nc.declare_dram_parameter("bpin", [1, M], dt.float32, isOutput=False)
    outT = nc.declare_dram_parameter("outT", [M, F], dt.float32, isOutput=True)

    # ---------------- internal DRAM ----------------
    xg0T = nc.dram_tensor("xg0T", [G4, F], dt.bfloat16)
    h0T = nc.dram_tensor("h0T", [H, F], dt.bfloat16)
    xg1T = nc.dram_tensor("xg1T", [G4, F], dt.bfloat16)
    h1T = nc.dram_tensor("h1T", [H, F], dt.bfloat16)

    ACT = mybir.ActivationFunctionType

    with tile.TileContext(nc) as tc:
        with tc.tile_pool(name="const", bufs=1) as cpool:
            b0sb = cpool.tile([128, 32], dt.float32, name="b0sb")
            b1sb = cpool.tile([128, 32], dt.float32, name="b1sb")
            bpsb = cpool.tile([M, 1], dt.float32, name="bpsb")
            # bias column m at b*sb[:, m]
            nc.sync.dma_start(b0sb[:], b0in[:].rearrange("o (m p) -> (o p) m", p=128))
            nc.sync.dma_start(b1sb[:], b1in[:].rearrange("o (m p) -> (o p) m", p=128))
            nc.sync.dma_start(bpsb[:], bpin[:].rearrange("o (m u) -> (o m) u", u=1))

            # persistent channel-major activations
            with tc.tile_pool(name="actsb", bufs=1) as apool:
                p2T = apool.tile([128, 2 * F], dt.bfloat16, name="p2T")

                # ---------- Ph2: prenet ----------
                with tc.tile_pool(name="pn", bufs=2) as pnp, \
                     tc.tile_pool(name="pnps", bufs=2, space="PSUM") as pnps:
                    prevT = pnp.tile([M, F], dt.float32, name="prevT")
                    nc.sync.dma_start(prevT[:], prevTin[:])
                    w1sb = pnp.tile([M, P], dt.float32, name="w1sb")
                    nc.sync.dma_start(w1sb[:], w1t[:])
                    p1T = pnp.tile([128, 2 * F], dt.float32, name="p1T")
                    for m in range(P // 128):
                        for n in range(NCHUNK):
                            ps = pnps.tile([128, FCH], dt.float32, name="pnps1", tag=f"pn{n % 4}")
                            nc.tensor.matmul(ps[:], w1sb[:, m * 128:(m + 1) * 128],
                                             prevT[:, n * FCH:(n + 1) * FCH], start=True, stop=True)
                            nc.scalar.activation(p1T[:, m * F + n * FCH: m * F + (n + 1) * FCH], ps[:], ACT.Relu)
                    w2sb = pnp.tile([128, 2 * P], dt.float32, name="w2sb")
                    for k in range(P // 128):
                        nc.sync.dma_start(w2sb[:, k * P:(k + 1) * P], w2t[k * 128:(k + 1) * 128, :])
                    for m in range(P // 128):
                        for n in range(NCHUNK):
                            ps = pnps.tile([128, FCH], dt.float32, name="pnps2", tag=f"pn{n % 4}")
                            for k in range(P // 128):
                                nc.tensor.matmul(ps[:], w2sb[:, k * P + m * 128: k * P + (m + 1) * 128],
                                                 p1T[:, k * F + n * FCH: k * F + (n + 1) * FCH],
                                                 start=(k == 0), stop=(k == 1))
                            nc.scalar.activation(p2T[:, m * F + n * FCH: m * F + (n + 1) * FCH], ps[:], ACT.Relu)

                # ---------- Ph3: xg0 ----------
                # rhs K-tiles: 2 from p2T, 4 from memT (SBUF-resident copy)
                KX = 6
                with tc.tile_pool(name="x0", bufs=2) as x0p, \
                     tc.tile_pool(name="x0ps", bufs=1, space="PSUM") as x0ps:
                    memTsb = x0p.tile([128, 4 * F], dt.bfloat16, name="memTsb")
                    for cb in range(4):
                        nc.sync.dma_start(memTsb[:, cb * F:(cb + 1) * F], memT[cb * 128:(cb + 1) * 128, :])

                    def x_rhs(k, n):
                        if k < 2:
                            return p2T[:, k * F + n * FCH: k * F + (n + 1) * FCH]
                        cb = k - 2
                        return memTsb[:, cb * F + n * FCH: cb * F + n * FCH + FCH]

                    for m in range(32):
                        wtile = x0p.tile([128, 6 * 128], dt.bfloat16, name="wtile", tag="w0t")
                        for k in range(KX):
                            nc.sync.dma_start(
                                wtile[:, k * 128:(k + 1) * 128],
                                wih0t[k * 128:(k + 1) * 128, m * 128:(m + 1) * 128])
                        pss = []
                        for n in range(NCHUNK):
                            ps = x0ps.tile([128, FCH], dt.float32, name="x0psn", tag=f"x0{n}")
                            pss.append(ps)
                        for k in range(KX):
                            for n in range(NCHUNK):
                                nc.tensor.matmul(pss[n][:], wtile[:, k * 128:(k + 1) * 128], x_rhs(k, n),
                                                 start=(k == 0), stop=(k == KX - 1))
                        for n in range(NCHUNK):
                            otile = x0p.tile([128, FCH], dt.bfloat16, name="otile", tag="x0o")
                            nc.vector.tensor_scalar_add(otile[:], pss[n][:], b0sb[:, m:m + 1])
                            nc.sync.dma_start(xg0T[m * 128:(m + 1) * 128, n * FCH:(n + 1) * FCH], otile[:])

            # ---------- recurrence helper ----------
            def recurrence(whhT_in, xgT_d, hT_out):
                SUB = 5                  # sub-blocks per loop body
                BODY = SUB * SBLK        # steps per loop body
                NB = T // BODY
                S4 = SBLK * 4
                with tc.tile_pool(name="rc", bufs=1) as rp, \
                     tc.tile_pool(name="rcx", bufs=2) as rxp, \
                     tc.tile_pool(name="rcps", bufs=1, space="PSUM") as rps, \
                     tc.tile_pool(name="rct", bufs=2) as rtp:
                    whsb = rp.tile([128, 8 * G4], dt.bfloat16, name="whsb")
                    for k in range(8):
                        nc.sync.dma_start(whsb[:, k * G4:(k + 1) * G4], whhT_in[k * 128:(k + 1) * 128, :])

                    def wh(k, mm):
                        return whsb[:, k * G4 + mm * 128: k * G4 + (mm + 1) * 128]

                    # h per half and sub-block, block-major: col blkloc*S4 + s*4 + b
                    hA = [rp.tile([128, 4 * S4], dt.bfloat16, name=f"hA{u}") for u in range(SUB)]
                    hB = [rp.tile([128, 4 * S4], dt.bfloat16, name=f"hB{u}") for u in range(SUB)]
                    cA = [rp.tile([128, 16], dt.float32, name=f"cA{i}") for i in range(2)]
                    cB = [rp.tile([128, 16], dt.float32, name=f"cB{i}") for i in range(2)]
                    for u in range(SUB):
                        nc.gpsimd.memset(hA[u][:], 0.0)
                        nc.gpsimd.memset(hB[u][:], 0.0)
                    for i in range(2):
                        nc.gpsimd.memset(cA[i][:], 0.0)
                        nc.gpsimd.memset(cB[i][:], 0.0)
                    # psum: full banks; psLo holds k0-3 partials for all 32 mm,
                    # psHiA/B hold k4-7 partials for mm 0-15 / 16-31
                    psLo = [rps.tile([128, 512], dt.float32, name=f"psLo{i}") for i in range(2)]
                    psHiA = [rps.tile([128, 512], dt.float32, name=f"psHiA{i}") for i in range(2)]
                    psHiB = [rps.tile([128, 512], dt.float32, name=f"psHiB{i}") for i in range(2)]

                    def rhs(hlist, g, kloc):
                        # h written at global step g (of BODY), read as rhs
                        u, sl = (g // SBLK) % SUB, g % SBLK
                        ht = hlist[u]
                        return ht[:, kloc * S4 + sl * 4: kloc * S4 + sl * 4 + 4]

                    def cell_pre(zt, psl, xg3):
                        # zt = psLo-part + xg   (off critical path)
                        z3 = zt[:].rearrange("p (m b) -> p m b", m=16)
                        nc.vector.tensor_add(z3, psl.rearrange("p (m b) -> p m b", m=16), xg3)

                    def cell_main(zt, psh, c_in, c_out, h_t, sl, tagc):
                        # zt += psHi; gates: cols gi*16+blk*4+b, gi in (i,f,o,g)
                        nc.vector.tensor_add(zt[:], zt[:], psh)
                        st = rtp.tile([128, 48], dt.float32, name="st", tag=f"st{tagc}")
                        nc.scalar.activation(st[:], zt[:, 0:48], ACT.Sigmoid)
                        gt = rtp.tile([128, 16], dt.float32, name="gt", tag=f"gt{tagc}")
                        nc.scalar.activation(gt[:], zt[:, 48:64], ACT.Tanh)
                        aa = rtp.tile([128, 16], dt.float32, name="aa", tag=f"aa{tagc}")
                        nc.vector.tensor_mul(aa[:], st[:, 16:32], c_in[:])
                        bb = rtp.tile([128, 16], dt.float32, name="bb", tag=f"bb{tagc}")
                        nc.vector.tensor_mul(bb[:], st[:, 0:16], gt[:])
                        nc.vector.tensor_add(c_out[:], aa[:], bb[:])
                        tcx = rtp.tile([128, 16], dt.float32, name="tcx", tag=f"tc{tagc}")
                        nc.scalar.activation(tcx[:], c_out[:], ACT.Tanh)
                        # h (bf16) into block-major slot sl
                        h3 = h_t[:].rearrange("p (blk sb) -> p blk sb", blk=4)[:, :, sl * 4:(sl + 1) * 4]
                        o3 = st[:, 32:48].rearrange("p (blk b) -> p blk b", blk=4)
                        t3 = tcx[:].rearrange("p (blk b) -> p blk b", blk=4)
                        nc.vector.tensor_mul(h3, o3, t3)

                    with tc.For_i(0, NB, 1, staggered_reset=True,
                                  hint_engines=(mybir.EngineType.PE,
                                                mybir.EngineType.DVE,
                                                mybir.EngineType.Activation)) as bi:
                        xgt = []
                        for u in range(SUB):
                            # xg cols rr*S4 + s*4 + b  (rr = m-tile)
                            xgsb = rxp.tile([128, 32 * S4], dt.bfloat16, name="xgsb", tag=f"xg{u}")
                            xgw = xgT_d[:, bass.ts(bi, SUB * S4)]
                            for rr in range(32):
                                nc.sync.dma_start(
                                    xgsb[:, rr * S4:(rr + 1) * S4],
                                    xgw[rr * 128:(rr + 1) * 128, u * S4:(u + 1) * S4])
                            xgt.append(xgsb[:].rearrange("p (rr sb) -> p rr sb", rr=32))

                        for g in range(BODY):
                            u, s = g // SBLK, g % SBLK
                            q = g % 2
                            xgv = xgt[u]
                            gp = (g - 1) % BODY
                            # balanced schedule: both cell chains get a ~3.2us
                            # window before their h is first consumed.
                            # F1: mm 0-15 kLo -> psLo[:, 0:64]
                            for mm in range(16):
                                dst = psLo[q][:, mm * 4: mm * 4 + 4]
                                for k in range(4):
                                    nc.tensor.matmul(dst, wh(k, mm), rhs(hA, gp, k),
                                                     start=(k == 0), stop=(k == 3))
                            ztA = rtp.tile([128, 64], dt.float32, name="ztA", tag=f"zt{q}A")
                            cell_pre(ztA, psLo[q][:, 0:64], xgv[:, 0:16, s * 4:(s + 1) * 4])
                            # F2: mm 16-23 kLo
                            for mm in range(16, 24):
                                dst = psLo[q][:, mm * 4: mm * 4 + 4]
                                for k in range(4):
                                    nc.tensor.matmul(dst, wh(k, mm), rhs(hA, gp, k),
                                                     start=(k == 0), stop=(k == 3))
                            # F3: mm 0-15 kHi -> psHiA
                            for mm in range(16):
                                dst = psHiA[q][:, mm * 4: mm * 4 + 4]
                                for k in range(4, 8):
                                    nc.tensor.matmul(dst, wh(k, mm), rhs(hB, gp, k - 4),
                                                     start=(k == 4), stop=(k == 7))
                            cell_main(ztA, psHiA[q][:, 0:64], cA[1 - q], cA[q],
                                      hA[u], s, f"{q}A")
                            # F4: mm 24-31 kLo
                            for mm in range(24, 32):
                                dst = psLo[q][:, mm * 4: mm * 4 + 4]
                                for k in range(4):
                                    nc.tensor.matmul(dst, wh(k, mm), rhs(hA, gp, k),
                                                     start=(k == 0), stop=(k == 3))
                            ztB = rtp.tile([128, 64], dt.float32, name="ztB", tag=f"zt{q}B")
                            cell_pre(ztB, psLo[q][:, 64:128], xgv[:, 16:32, s * 4:(s + 1) * 4])
                            # F5: mm 16-31 kHi -> psHiB
                            for mm in range(16, 32):
                                dst = psHiB[q][:, (mm - 16) * 4:(mm - 16) * 4 + 4]
                                for k in range(4, 8):
                                    nc.tensor.matmul(dst, wh(k, mm), rhs(hB, gp, k - 4),
                                                     start=(k == 4), stop=(k == 7))
                            cell_main(ztB, psHiB[q][:, 0:64], cB[1 - q], cB[q],
                                      hB[u], s, f"{q}B")

                            if s == SBLK - 1:
                                # contiguous h stores for this sub-block (scalar
                                # queue, so sync-queue xg loads are not blocked)
                                hw = hT_out[:, bass.ts(bi, SUB * S4)]
                                for half, ht in ((0, hA[u]), (1, hB[u])):
                                    for bl in range(4):
                                        gb = half * 4 + bl
                                        nc.scalar.dma_start(
                                            hw[gb * 128:(gb + 1) * 128, u * S4:(u + 1) * S4],
                                            ht[:, bl * S4:(bl + 1) * S4])

            # ---------- Ph4: layer-0 recurrence ----------
            recurrence(whh0t, xg0T, h0T)

            # ---------- Ph5: xg1 ----------
            with tc.tile_pool(name="x1", bufs=1) as x1p, \
                 tc.tile_pool(name="x1w", bufs=2) as x1wp, \
                 tc.tile_pool(name="x1ps", bufs=1, space="PSUM") as x1ps:
                h0sb = x1p.tile([128, 8 * F], dt.bfloat16, name="h0sb")
                for k in range(8):
                    nc.sync.dma_start(h0sb[:, k * F:(k + 1) * F], h0T[k * 128:(k + 1) * 128, :])
                for m in range(32):
                    wtile = x1wp.tile([128, 8 * 128], dt.bfloat16, name="w1tile", tag="w1t")
                    for k in range(8):
                        nc.sync.dma_start(
                            wtile[:, k * 128:(k + 1) * 128],
                            wih1t[k * 128:(k + 1) * 128, m * 128:(m + 1) * 128])
                    pss = []
                    for n in range(NCHUNK):
                        ps = x1ps.tile([128, FCH], dt.float32, name="x1psn", tag=f"x1{n}")
                        pss.append(ps)
                    for k in range(8):
                        for n in range(NCHUNK):
                            nc.tensor.matmul(pss[n][:], wtile[:, k * 128:(k + 1) * 128],
                                             h0sb[:, k * F + n * FCH: k * F + n * FCH + FCH],
                                             start=(k == 0), stop=(k == 7))
                    for n in range(NCHUNK):
                        otile = x1wp.tile([128, FCH], dt.bfloat16, name="o1tile", tag="x1o")
                        nc.vector.tensor_scalar_add(otile[:], pss[n][:], b1sb[:, m:m + 1])
                        nc.sync.dma_start(xg1T[m * 128:(m + 1) * 128, n * FCH:(n + 1) * FCH], otile[:])

            # ---------- Ph6: layer-1 recurrence ----------
            recurrence(whh1t, xg1T, h1T)

            # ---------- Ph7: projection ----------
            with tc.tile_pool(name="pj", bufs=1) as pjp, \
                 tc.tile_pool(name="pjw", bufs=2) as pjwp, \
                 tc.tile_pool(name="pjps", bufs=2, space="PSUM") as pjps:
                h1sb = pjp.tile([128, 8 * F], dt.bfloat16, name="h1sb")
                for k in range(8):
                    nc.sync.dma_start(h1sb[:, k * F:(k + 1) * F], h1T[k * 128:(k + 1) * 128, :])
                memTsb2 = pjp.tile([128, 4 * F], dt.bfloat16, name="memTsb2")
                for cb in range(4):
                    nc.sync.dma_start(memTsb2[:, cb * F:(cb + 1) * F], memT[cb * 128:(cb + 1) * 128, :])
                wphsb = pjp.tile([128, 8 * M], dt.bfloat16, name="wphsb")
                for k in range(8):
                    nc.sync.dma_start(wphsb[:, k * M:(k + 1) * M], wpt_h[k * 128:(k + 1) * 128, :])
                wpmsb = pjp.tile([128, 4 * M], dt.bfloat16, name="wpmsb")
                for k in range(4):
                    nc.sync.dma_start(wpmsb[:, k * M:(k + 1) * M], wpt_m[k * 128:(k + 1) * 128, :])
                for n in range(NCHUNK):
                    ps = pjps.tile([M, FCH], dt.float32, name="pjpsn", tag=f"pj{n % 4}")
                    for k in range(8):
                        nc.tensor.matmul(ps[:], wphsb[:, k * M:(k + 1) * M],
                                         h1sb[:, k * F + n * FCH: k * F + n * FCH + FCH],
                                         start=(k == 0), stop=False)
                    for cb in range(4):
                        nc.tensor.matmul(ps[:], wpmsb[:, cb * M:(cb + 1) * M],
                                         memTsb2[:, cb * F + n * FCH: cb * F + n * FCH + FCH],
                                         start=False, stop=(cb == 3))
                    otile = pjwp.tile([M, FCH], dt.float32, name="pjo", tag="pjo")
                    nc.vector.tensor_scalar_add(otile[:], ps[:], bpsb[:, 0:1])
                    nc.sync.dma_start(outT[:, n * FCH:(n + 1) * FCH], otile[:])

    nc.finalize()
    return nc


def kernel(memory, y_mels, W1, W2, w_ih0, w_hh0, b_ih0, b_hh0,
           w_ih1, w_hh1, b_ih1, b_hh1, W_proj, b_proj, _trace=False):
    from concourse.bass_utils import run_bass_kernel_spmd

    nc = _build()
    bf16 = ml_dtypes.bfloat16
    w1t = np.ascontiguousarray(W1.T.astype(np.float32))
    w2t = np.ascontiguousarray(W2.T.astype(np.float32))
    wih0t = _arrange_cols(w_ih0.T.astype(np.float32)).astype(bf16)
    whh0t = _arrange_cols(w_hh0.T.astype(np.float32)).astype(bf16)
    wih1t = _arrange_cols(w_ih1.T.astype(np.float32)).astype(bf16)
    whh1t = _arrange_cols(w_hh1.T.astype(np.float32)).astype(bf16)
    b0 = _arrange_vec((b_ih0 + b_hh0).astype(np.float32)).reshape(1, G4)
    b1 = _arrange_vec((b_ih1 + b_hh1).astype(np.float32)).reshape(1, G4)
    wpt = W_proj.T.astype(np.float32)
    wpt_h = np.ascontiguousarray(wpt[:H]).astype(bf16)
    wpt_m = np.ascontiguousarray(wpt[H:]).astype(bf16)
    bp = b_proj.astype(np.float32).reshape(1, M)

    in_maps = []
    for c in range(NCORES):
        mem_c = memory[c * BC:(c + 1) * BC]          # [4, 1000, 512]
        y_c = y_mels[c * BC:(c + 1) * BC]            # [4, 1000, 80]
        # channel-major [A, F] / shifted mels [M, F], frame f = t*BC + b
        memT_c = np.ascontiguousarray(
            mem_c.transpose(2, 1, 0).reshape(A, F).astype(bf16))
        prev_c = np.concatenate(
            [np.zeros((BC, 1, M), np.float32), y_c[:, :-1, :]], axis=1)
        prevT_c = np.ascontiguousarray(
            prev_c.transpose(2, 1, 0).reshape(M, F).astype(np.float32))
        in_maps.append(dict(
            memT=memT_c, prevTin=prevT_c, w1t=w1t, w2t=w2t,
            wih0t=wih0t, whh0t=whh0t, wih1t=wih1t, whh1t=whh1t,
            b0in=b0, b1in=b1, wpt_h=wpt_h, wpt_m=wpt_m, bpin=bp))

    res = run_bass_kernel_spmd(nc, in_maps, core_ids=list(range(NCORES)), trace=_trace)
    outs = []
    for c in range(NCORES):
        oT = res.results[c]["outT"]                  # [80, 4000]
        outs.append(oT.reshape(M, T, BC).transpose(2, 1, 0))  # [4, 1000, 80]
    full = np.concatenate(outs, axis=0).astype(np.float32)
    if _trace:
        kernel.last_exec_time_ns = res.exec_time_ns
    return full
